# revision 3
# baseline (speedup 1.0000x reference)
"""Multi-head attention (B=2, S=2048, D=1024, H=16, dk=64) on 8 Trainium2
NeuronCores via Bass/Tile.

Sharding: core c handles batch b = c//4 and head-group g = c%4 (4 heads,
256 qkv columns).  Each core computes its QKV projection slices, 4 heads of
attention, and a partial output projection against its 256-row slice of Wo.
The host sums the 4 partial outputs per batch (row-sharded Wo => partial
sums) and folds in the biases bo and bv@Wo (softmax rows sum to 1, so the
V-bias contributes exactly bv@Wo per token).

On-device layout notes:
- All matmuls run in float32r (full-rate fp32 PE mode, ~1.5e-4 rounding).
- scoresT is computed as [k_tok, q_tok] so the AV matmul needs no transpose
  of the softmax matrix; softmax normalization happens after AV on the much
  smaller O matrix via a PE transpose (which also carries the exp-sums row).
- Ṽ carries a ones column per head so AV's PSUM accumulation also produces
  the softmax denominators (row 64 of each head's [65, q] output).
"""

import numpy as np

P = 128
B, S, D = 2, 2048, 1024
H, DK = 16, 64
COLS = 256          # qkv columns per core (4 heads)
KC = D // P         # 8 contraction chunks for the projections
TT = 512            # token block (matmul free dim)
NJ = S // TT        # 4 token blocks
NT = S // P         # 16 token tiles
NKT = S // P        # 16 key tiles
VW = 65             # per-head AV lhsT width: 64 v-dims + ones column

_CACHE = {}


def _build():
    import concourse.bass as bass
    import concourse.tile as tile
    from concourse import bacc, mybir
    from concourse.masks import make_identity

    f32 = mybir.dt.float32
    f32r = mybir.dt.float32r
    Exp = mybir.ActivationFunctionType.Exp

    nc = bacc.Bacc(
        "TRN2", target_bir_lowering=False, debug=False,
        enable_asserts=False, num_devices=8,
    )
    x_d = nc.dram_tensor("x", [S, D], f32, kind="ExternalInput").ap()
    wq_d = nc.dram_tensor("wq", [D, COLS], f32, kind="ExternalInput").ap()
    wk_d = nc.dram_tensor("wk", [D, COLS], f32, kind="ExternalInput").ap()
    wv_d = nc.dram_tensor("wv", [D, COLS], f32, kind="ExternalInput").ap()
    wo_d = nc.dram_tensor("wo", [COLS, D], f32, kind="ExternalInput").ap()
    bq_d = nc.dram_tensor("bq", [COLS], f32, kind="ExternalInput").ap()
    bk_d = nc.dram_tensor("bk", [COLS], f32, kind="ExternalInput").ap()
    out_d = nc.dram_tensor("out_t", [D, S], f32, kind="ExternalOutput").ap()

    with tile.TileContext(nc) as tc:
        with (
            tc.tile_pool(name="const", bufs=1) as const,
            tc.tile_pool(name="wpool", bufs=1) as wpool,
            tc.tile_pool(name="persist", bufs=1) as persist,
            tc.tile_pool(name="xin", bufs=3) as xin,
            tc.tile_pool(name="xtp", bufs=2) as xtp,
            tc.tile_pool(name="exps", bufs=6) as exps,
            tc.tile_pool(name="stage", bufs=4) as stage,
            tc.tile_pool(name="outst", bufs=4) as outst,
            tc.tile_pool(name="ps_mm", bufs=3, space="PSUM") as ps_mm,
            tc.tile_pool(name="ps_acc", bufs=3, space="PSUM") as ps_acc,
            tc.tile_pool(name="ps_tr", bufs=2, space="PSUM") as ps_tr,
        ):
            ident = const.tile([P, P], f32)
            make_identity(nc, ident)

            # ---- weights: DMA fp32 -> convert to f32r on DVE ----
            def load_w(dram, shape_free, name):
                st = const.tile([P, KC, shape_free], f32, tag="wstage")
                nc.sync.dma_start(st[:], dram.rearrange("(o p) f -> p o f", p=P))
                wr = wpool.tile([P, KC, shape_free], f32r, tag=f"w_{name}")
                nc.vector.tensor_copy(wr[:], st[:])
                return wr

            wq_r = load_w(wq_d, COLS, "q")
            wk_r = load_w(wk_d, COLS, "k")
            wv_r = load_w(wv_d, COLS, "v")
            wo_st = const.tile([P, 2, D], f32, tag="wostage")
            nc.sync.dma_start(wo_st[:], wo_d.rearrange("(o p) f -> p o f", p=P))
            wo_r = wpool.tile([P, 2, D], f32r, tag="w_o")
            nc.vector.tensor_copy(wo_r[:], wo_st[:])

            bq_sb = const.tile([P, 2], f32, tag="bq")
            nc.sync.dma_start(bq_sb[:], bq_d.rearrange("(o p) -> p o", p=P))
            bk_sb = const.tile([P, 2], f32, tag="bk")
            nc.sync.dma_start(bk_sb[:], bk_d.rearrange("(o p) -> p o", p=P))

            # persistent activations
            qT = persist.tile([P, 2, S], f32r, tag="qT")    # [qcol, tok]
            kT = persist.tile([P, 2, S], f32r, tag="kT")    # [kcol, tok]
            vt = persist.tile([P, NT, 4 * VW], f32r, tag="vt")  # [tok, head*65]
            oT = persist.tile([P, 2, S], f32r, tag="oT")    # [vdim, tok]

            # ones columns of Ṽ
            ones_st = const.tile([P, NT * 4], f32, tag="ones")
            nc.vector.memset(ones_st[:], 1.0)
            vt_heads = vt[:].rearrange("p t (h c) -> p t h c", c=VW)
            nc.vector.tensor_copy(vt_heads[:, :, :, 64], ones_st[:])

            # ---- phase 0/1: x transpose + QKV projections, per token block ----
            for j in range(NJ):
                xts = []
                for ts in range(TT // P):
                    xt = xin.tile([P, D], f32, tag="xin")
                    nc.sync.dma_start(xt[:], x_d[bass.ts(4 * j + ts, P), :])
                    xts.append(xt)

                xT = xtp.tile([P, KC, TT], f32r, tag="xT")
                for ts in range(TT // P):
                    for kc in range(KC):
                        tr = ps_tr.tile([P, P], f32, tag="tr")
                        nc.tensor.transpose(tr[:], xts[ts][:, bass.ts(kc, P)], ident[:])
                        nc.vector.tensor_copy(xT[:, kc, bass.ts(ts, P)], tr[:])

                # Q^T, K^T: [qcol, tok] with bias
                for (wmat, bsb, dstT) in ((wq_r, bq_sb, qT), (wk_r, bk_sb, kT)):
                    for ct in range(2):
                        acc = ps_mm.tile([P, TT], f32, tag="mm")
                        for kc in range(KC):
                            nc.tensor.matmul(
                                acc[:], wmat[:, kc, bass.ts(ct, P)], xT[:, kc, :],
                                start=(kc == 0), stop=(kc == KC - 1),
                            )
                        nc.vector.tensor_scalar_add(
                            dstT[:, ct, bass.ts(j, TT)], acc[:], bsb[:, ct : ct + 1]
                        )

                # V: [tok, vcol]
                for ts in range(TT // P):
                    acc = ps_mm.tile([P, COLS], f32, tag="mm")
                    for kc in range(KC):
                        nc.tensor.matmul(
                            acc[:], xT[:, kc, bass.ts(ts, P)], wv_r[:, kc, :],
                            start=(kc == 0), stop=(kc == KC - 1),
                        )
                    tt = 4 * j + ts
                    nc.vector.tensor_copy(
                        vt_heads[:, tt, :, 0:64],
                        acc[:].rearrange("p (h c) -> p h c", c=64),
                    )

            # ---- phase 2: attention per head-pair p and query block j ----
            for p in range(2):
                for j in range(NJ):
                    o_ps = [
                        ps_acc.tile([VW, TT], f32, tag="acc", name=f"o_ps{i}")
                        for i in range(2)
                    ]
                    for kc in range(NKT):
                        for i in range(2):
                            lo, hi = 64 * i, 64 * i + 64
                            sc = ps_mm.tile([P, TT], f32, tag="mm")
                            nc.tensor.matmul(
                                sc[:],
                                kT[lo:hi, p, bass.ts(kc, P)],
                                qT[lo:hi, p, bass.ts(j, TT)],
                                start=True, stop=True,
                            )
                            ex = exps.tile([P, TT], f32r, tag="exp")
                            nc.scalar.activation(ex[:], sc[:], Exp, scale=0.125)
                            h = 2 * p + i
                            nc.tensor.matmul(
                                o_ps[i][:],
                                vt[:, kc, bass.ds(VW * h, VW)],
                                ex[:],
                                start=(kc == 0), stop=(kc == NKT - 1),
                            )

                    # normalize + build O^T
                    for i in range(2):
                        o_st = stage.tile([VW, TT], f32, tag="ost")
                        nc.vector.tensor_copy(o_st[:], o_ps[i][:])
                        hm = 64 * i
                        for ts in range(TT // P):
                            trp = ps_tr.tile([P, VW], f32, tag="tr")
                            nc.tensor.transpose(
                                trp[:], o_st[:, bass.ts(ts, P)], ident[:VW, :VW]
                            )
                            rec = stage.tile([P, 1], f32, tag="rec")
                            nc.vector.reciprocal(rec[:], trp[:, 64:65])
                            onq = stage.tile([P, 64], f32, tag="onq")
                            nc.vector.tensor_scalar_mul(onq[:], trp[:, 0:64], rec[:])
                            trb = ps_tr.tile([64, P], f32, tag="tr")
                            nc.tensor.transpose(trb[:], onq[:], ident[:])
                            nc.vector.tensor_copy(
                                oT[hm : hm + 64, p, bass.ds(j * TT + ts * P, P)],
                                trb[:],
                            )

            # ---- phase 3: partial output projection out^T = Wo_slice^T @ O ----
            for oc in range(D // P):
                for j in range(NJ):
                    acc = ps_acc.tile([P, TT], f32, tag="acc")
                    for vc in range(2):
                        nc.tensor.matmul(
                            acc[:], wo_r[:, vc, bass.ts(oc, P)], oT[:, vc, bass.ts(j, TT)],
                            start=(vc == 0), stop=(vc == 1),
                        )
                    st = outst.tile([P, TT], f32, tag="outst")
                    nc.vector.tensor_copy(st[:], acc[:])
                    nc.sync.dma_start(out_d[bass.ts(oc, P), bass.ts(j, TT)], st[:])

    nc.compile()
    return nc


def kernel(x, Wq, bq, Wk, bk, Wv, bv, Wo, bo):
    from concourse import bass_utils

    x = np.asarray(x, dtype=np.float32)
    Wq = np.asarray(Wq, dtype=np.float32)
    Wk = np.asarray(Wk, dtype=np.float32)
    Wv = np.asarray(Wv, dtype=np.float32)
    Wo = np.asarray(Wo, dtype=np.float32)
    bq = np.asarray(bq, dtype=np.float32)
    bk = np.asarray(bk, dtype=np.float32)
    bv = np.asarray(bv, dtype=np.float32)
    bo = np.asarray(bo, dtype=np.float32)

    if "nc" not in _CACHE:
        _CACHE["nc"] = _build()
    nc = _CACHE["nc"]

    in_maps = []
    for c in range(8):
        b, g = divmod(c, 4)
        cs = slice(COLS * g, COLS * (g + 1))
        in_maps.append({
            "x": np.ascontiguousarray(x[b]),
            "wq": np.ascontiguousarray(Wq[:, cs]),
            "wk": np.ascontiguousarray(Wk[:, cs]),
            "wv": np.ascontiguousarray(Wv[:, cs]),
            "wo": np.ascontiguousarray(Wo[cs, :]),
            "bq": np.ascontiguousarray(bq[cs]),
            "bk": np.ascontiguousarray(bk[cs]),
        })

    res = bass_utils.run_bass_kernel_spmd(nc, in_maps, core_ids=list(range(8)))

    out = np.zeros((B, S, D), dtype=np.float32)
    for c in range(8):
        out[c // 4] += res.results[c]["out_t"].T
    out += bo + bv @ Wo
    return out


# revision 11
# speedup vs baseline: 1.0157x; 1.0157x over previous
"""Multi-head attention (B=2, S=2048, D=1024, H=16, dk=64) on 8 Trainium2
NeuronCores via Bass/Tile.

Sharding: core c handles batch b = c//4 and head-group g = c%4 (4 heads,
256 qkv columns).  Each core computes its QKV projection slices, 4 heads of
attention, and a partial output projection against its 256-row slice of Wo.
The host sums the 4 partial outputs per batch (row-sharded Wo => partial
sums) and folds in the biases bo and bv@Wo (softmax rows sum to 1, so the
V-bias contributes exactly bv@Wo per token).

v2 design notes:
- All matmuls in float32r (full-rate fp32 PE mode, ~1.5e-4 rounding); PE
  transposes also f32r (1.5 cyc/row) to avoid dtype switches.
- scoresT [k_tok, q_tok] per head via K=64 row-packed head pairs
  (tile_position (0,0)/(64,0) derived from base partitions) -> concurrent.
- Scores land in one shared 4-bank PSUM tensor [128, 8, 512]; ONE ACT exp
  per 2 k-chunks covers [128, 2048] (amortizes the 352-cycle ACT overhead).
- AV lhsT = [1 | V_h] so PSUM row 0 accumulates the softmax denominators.
- Normalization without PE transposes: DVE reciprocal of the sums row,
  PE ones-outer-product broadcast to [65, 512], DVE multiply, then a
  partition-shifting SBUF->SBUF DMA routes each head into O^T layout.
"""

import numpy as np

P = 128
B, S, D = 2, 2048, 1024
H, DK = 16, 64
COLS = 256          # qkv columns per core (4 heads)
KC = D // P         # 8 contraction chunks for the projections
TT = 512            # token block (matmul free dim)
NJ = S // TT        # 4 token blocks
NT = S // P         # 16 token tiles
NKT = S // P        # 16 key tiles
VW = 65             # per-head AV lhsT width: ones column + 64 v-dims

_CACHE = {}


def _build():
    import concourse.bass as bass
    import concourse.tile as tile
    from concourse import bacc, mybir
    from concourse.masks import make_identity

    f32 = mybir.dt.float32
    f32r = mybir.dt.float32r
    Exp = mybir.ActivationFunctionType.Exp

    nc = bacc.Bacc(
        "TRN2", target_bir_lowering=False, debug=False,
        enable_asserts=False, num_devices=8,
    )
    x_d = nc.dram_tensor("x", [S, D], f32, kind="ExternalInput").ap()
    wq_d = nc.dram_tensor("wq", [D, COLS], f32, kind="ExternalInput").ap()
    wk_d = nc.dram_tensor("wk", [D, COLS], f32, kind="ExternalInput").ap()
    wv_d = nc.dram_tensor("wv", [D, COLS], f32, kind="ExternalInput").ap()
    wo_d = nc.dram_tensor("wo", [COLS, D], f32, kind="ExternalInput").ap()
    bq_d = nc.dram_tensor("bq", [COLS], f32, kind="ExternalInput").ap()
    bk_d = nc.dram_tensor("bk", [COLS], f32, kind="ExternalInput").ap()
    out_d = nc.dram_tensor("out_t", [D, S], f32, kind="ExternalOutput").ap()

    with tile.TileContext(nc) as tc:
        with (
            tc.tile_pool(name="const", bufs=1) as const,
            tc.tile_pool(name="wst", bufs=1) as wst,
            tc.tile_pool(name="wpool", bufs=1) as wpool,
            tc.tile_pool(name="persist", bufs=1) as persist,
            tc.tile_pool(name="xin", bufs=2) as xin,
            tc.tile_pool(name="xrp", bufs=2) as xrp,
            tc.tile_pool(name="xtp", bufs=2) as xtp,
            tc.tile_pool(name="exps", bufs=2) as exps,
            tc.tile_pool(name="stage", bufs=3) as stage,
            tc.tile_pool(name="outst", bufs=4) as outst,
            tc.tile_pool(name="ps_sc", bufs=1, space="PSUM") as ps_sc,
            tc.tile_pool(name="ps_acc", bufs=2, space="PSUM") as ps_acc,
            tc.tile_pool(name="ps_u", bufs=2, space="PSUM") as ps_u,
        ):
            ident = const.tile([P, P], f32)
            make_identity(nc, ident)
            ident_r = const.tile([P, P], f32r)
            nc.vector.tensor_copy(ident_r[:], ident[:])

            ones32 = const.tile([P, VW], f32, tag="ones32")
            nc.vector.memset(ones32[:], 1.0)
            ones_r = const.tile([P, VW], f32r, tag="ones_r")
            nc.vector.tensor_copy(ones_r[:], ones32[:])

            # ---- weights: DMA fp32 -> convert to f32r on DVE ----
            def load_w(dram, shape_free, name):
                st = wst.tile([P, KC, shape_free], f32, tag="wstage", name="wstage")
                nc.sync.dma_start(st[:], dram.rearrange("(o p) f -> p o f", p=P))
                wr = wpool.tile([P, KC, shape_free], f32r, tag=f"w_{name}",
                                name=f"w_{name}")
                nc.vector.tensor_copy(wr[:], st[:])
                return wr

            wq_r = load_w(wq_d, COLS, "q")
            wk_r = load_w(wk_d, COLS, "k")
            wv_r = load_w(wv_d, COLS, "v")
            wo_st = wst.tile([P, 2, D], f32, tag="wstage", name="wostage")
            nc.sync.dma_start(wo_st[:], wo_d.rearrange("(o p) f -> p o f", p=P))
            wo_r = wpool.tile([P, 2, D], f32r, tag="w_o")
            nc.vector.tensor_copy(wo_r[:], wo_st[:])

            bq_sb = const.tile([P, 2], f32, tag="bq")
            nc.sync.dma_start(bq_sb[:], bq_d.rearrange("(o p) -> p o", p=P))
            bk_sb = const.tile([P, 2], f32, tag="bk")
            nc.sync.dma_start(bk_sb[:], bk_d.rearrange("(o p) -> p o", p=P))

            # persistent activations
            qT = persist.tile([P, 2, S], f32r, tag="qT")    # [qcol, tok]
            kT = persist.tile([P, 2, S], f32r, tag="kT")    # [kcol, tok]
            vt = persist.tile([P, NT, 4 * VW], f32r, tag="vt")  # [tok, h*(1|V)]
            oT = persist.tile([P, 2, S], f32r, tag="oT")    # [vdim, tok]

            # ones column (index 64 of each head's VW slice)
            vt_heads = vt[:].rearrange("p t (h c) -> p t h c", c=VW)
            nc.vector.tensor_copy(
                vt_heads[:, :, :, 64],
                ones32[:, :NT * 4].rearrange("p (t h) -> p t h", h=4),
            )

            # ---- phase 0/1: x transpose + QKV projections, per token block ----
            for j in range(NJ):
                xT = xtp.tile([P, KC, TT], f32r, tag="xT")
                for ts in range(TT // P):
                    xt = xin.tile([P, D], f32, tag="xin", name="xt")
                    nc.sync.dma_start(xt[:], x_d[bass.ts(4 * j + ts, P), :])
                    xr = xrp.tile([P, D], f32r, tag="xr", name="xr")
                    nc.vector.tensor_copy(xr[:], xt[:])
                    for kh in range(2):
                        tr = ps_u.tile([P, 4, P], f32r, tag="u", name="tru")
                        for k4 in range(4):
                            nc.tensor.transpose(
                                tr[:, k4, :],
                                xr[:, bass.ts(4 * kh + k4, P)],
                                ident_r[:],
                            )
                        nc.vector.tensor_copy(
                            xT[:, 4 * kh : 4 * kh + 4, bass.ds(ts * P, P)], tr[:]
                        )

                # Q^T, K^T: [qcol, tok] with bias
                for (wmat, bsb, dstT) in ((wq_r, bq_sb, qT), (wk_r, bk_sb, kT)):
                    for ct in range(2):
                        acc = ps_u.tile([P, TT], f32, tag="u", name="qk_acc")
                        for kc in range(KC):
                            nc.tensor.matmul(
                                acc[:], wmat[:, kc, bass.ts(ct, P)], xT[:, kc, :],
                                start=(kc == 0), stop=(kc == KC - 1),
                            )
                        nc.vector.tensor_scalar_add(
                            dstT[:, ct, bass.ts(j, TT)], acc[:], bsb[:, ct : ct + 1]
                        )

                # V: [tok, vcol]
                for ts in range(TT // P):
                    acc = ps_u.tile([P, COLS], f32, tag="u", name="v_acc")
                    for kc in range(KC):
                        nc.tensor.matmul(
                            acc[:], xT[:, kc, bass.ts(ts, P)], wv_r[:, kc, :],
                            start=(kc == 0), stop=(kc == KC - 1),
                        )
                    tt = 4 * j + ts
                    nc.vector.tensor_copy(
                        vt_heads[:, tt, :, 0:64],
                        acc[:].rearrange("p (h c) -> p h c", c=64),
                    )

            # shared scores PSUM tensor: 4 slots x [128, 512] = 4 banks
            big_sc = ps_sc.tile([P, 4, TT], f32, tag="sc")

            # ---- phase 2 + 3 interleaved over token blocks ----
            for j in range(NJ):
                for p in range(2):
                    o_ps = [
                        ps_acc.tile([VW, TT], f32, tag="acc", name=f"o_ps{i}")
                        for i in range(2)
                    ]
                    for kc in range(NKT):
                        base = (2 * kc) % 4
                        for i in range(2):
                            lo, hi = 64 * i, 64 * i + 64
                            nc.tensor.matmul(
                                big_sc[:, base + i, :],
                                kT[lo:hi, p, bass.ts(kc, P)],
                                qT[lo:hi, p, bass.ts(j, TT)],
                                start=True, stop=True,
                            )
                        ex = exps.tile([P, 2, TT], f32r, tag="exp", name="ex")
                        nc.scalar.activation(
                            ex[:], big_sc[:, base : base + 2, :], Exp,
                            scale=0.125,
                        )
                        for i in range(2):
                            h = 2 * p + i
                            nc.tensor.matmul(
                                o_ps[i][:],
                                vt[:, kc, bass.ds(VW * h, VW)],
                                ex[:, i, :],
                                start=(kc == 0), stop=(kc == NKT - 1),
                            )

                    # normalize both heads into O^T via recip/broadcast/mult/DMA
                    o32 = stage.tile([P, TT], f32, tag="o32", name="o32")
                    for i in range(2):
                        rrow = stage.tile([P, TT], f32r, tag="rrow", name="rrow")
                        with nc.allow_low_precision(reason="f32r recip for matmul"):
                            nc.vector.reciprocal(
                                rrow[64:65, :], o_ps[i][64:65, :]
                            )
                        rbc = ps_u.tile([64, TT], f32, tag="u", name="rbc")
                        nc.tensor.matmul(
                            rbc[:], ones_r[64:65, 0:64], rrow[64:65, :],
                            start=True, stop=True,
                        )
                        rbs = stage.tile([64, TT], f32, tag="rbs", name="rbs")
                        nc.vector.tensor_copy(rbs[:], rbc[:])
                        onrm = stage.tile([P, TT], f32, tag="onrm", name="onrm")
                        nc.vector.tensor_tensor(
                            onrm[0:64, :], o_ps[i][0:64, :], rbs[:],
                            mybir.AluOpType.mult,
                        )
                        nc.sync.dma_start(
                            o32[bass.ds(64 * i, 64), :], onrm[0:64, :]
                        )
                    nc.vector.tensor_copy(oT[:, p, bass.ts(j, TT)], o32[:])

                # partial output projection for this token block
                for oc in range(D // P):
                    acc = ps_u.tile([P, TT], f32, tag="u", name="wo_acc")
                    for vc in range(2):
                        nc.tensor.matmul(
                            acc[:], wo_r[:, vc, bass.ts(oc, P)],
                            oT[:, vc, bass.ts(j, TT)],
                            start=(vc == 0), stop=(vc == 1),
                        )
                    st = outst.tile([P, TT], f32, tag="outst", name="outst")
                    nc.vector.tensor_copy(st[:], acc[:])
                    nc.sync.dma_start(out_d[bass.ts(oc, P), bass.ts(j, TT)], st[:])

    nc.compile()
    return nc


def kernel(x, Wq, bq, Wk, bk, Wv, bv, Wo, bo):
    from concourse import bass_utils

    x = np.asarray(x, dtype=np.float32)
    Wq = np.asarray(Wq, dtype=np.float32)
    Wk = np.asarray(Wk, dtype=np.float32)
    Wv = np.asarray(Wv, dtype=np.float32)
    Wo = np.asarray(Wo, dtype=np.float32)
    bq = np.asarray(bq, dtype=np.float32)
    bk = np.asarray(bk, dtype=np.float32)
    bv = np.asarray(bv, dtype=np.float32)
    bo = np.asarray(bo, dtype=np.float32)

    if "nc" not in _CACHE:
        _CACHE["nc"] = _build()
    nc = _CACHE["nc"]

    in_maps = []
    for c in range(8):
        b, g = divmod(c, 4)
        cs = slice(COLS * g, COLS * (g + 1))
        in_maps.append({
            "x": np.ascontiguousarray(x[b]),
            "wq": np.ascontiguousarray(Wq[:, cs]),
            "wk": np.ascontiguousarray(Wk[:, cs]),
            "wv": np.ascontiguousarray(Wv[:, cs]),
            "wo": np.ascontiguousarray(Wo[cs, :]),
            "bq": np.ascontiguousarray(bq[cs]),
            "bk": np.ascontiguousarray(bk[cs]),
        })

    res = bass_utils.run_bass_kernel_spmd(nc, in_maps, core_ids=list(range(8)))

    out = np.zeros((B, S, D), dtype=np.float32)
    for c in range(8):
        out[c // 4] += res.results[c]["out_t"].T
    out += bo + bv @ Wo
    return out


# revision 19
# speedup vs baseline: 1.0418x; 1.0257x over previous
"""Multi-head attention (B=2, S=2048, D=1024, H=16, dk=64) on 8 Trainium2
NeuronCores via Bass/Tile.

Sharding: core c handles batch b = c//4 and head-group g = c%4 (4 heads,
256 qkv columns).  Each core computes its QKV projection slices, 4 heads of
attention, and a partial output projection against its 256-row slice of Wo.
The host sums the 4 partial outputs per batch (row-sharded Wo => partial
sums) and folds in the biases bo and bv@Wo (softmax rows sum to 1, so the
V-bias contributes exactly bv@Wo per token).

v2 design notes:
- All matmuls in float32r (full-rate fp32 PE mode, ~1.5e-4 rounding); PE
  transposes also f32r (1.5 cyc/row) to avoid dtype switches.
- scoresT [k_tok, q_tok] per head via K=64 row-packed head pairs
  (tile_position (0,0)/(64,0) derived from base partitions) -> concurrent.
- Scores land in one shared 4-bank PSUM tensor [128, 8, 512]; ONE ACT exp
  per 2 k-chunks covers [128, 2048] (amortizes the 352-cycle ACT overhead).
- AV lhsT = [1 | V_h] so PSUM row 0 accumulates the softmax denominators.
- Normalization without PE transposes: DVE reciprocal of the sums row,
  PE ones-outer-product broadcast to [65, 512], DVE multiply, then a
  partition-shifting SBUF->SBUF DMA routes each head into O^T layout.
"""

import numpy as np

P = 128
B, S, D = 2, 2048, 1024
H, DK = 16, 64
COLS = 256          # qkv columns per core (4 heads)
KC = D // P         # 8 contraction chunks for the projections
TT = 512            # token block (matmul free dim)
NJ = S // TT        # 4 token blocks
NT = S // P         # 16 token tiles
NKT = S // P        # 16 key tiles
VW = 65             # per-head AV lhsT width: ones column + 64 v-dims

_CACHE = {}


def _build():
    import concourse.bass as bass
    import concourse.tile as tile
    from concourse import bacc, mybir

    f32 = mybir.dt.float32
    f32r = mybir.dt.float32r
    Exp = mybir.ActivationFunctionType.Exp

    bf16 = mybir.dt.bfloat16
    nc = bacc.Bacc(
        "TRN2", target_bir_lowering=False, debug=False,
        enable_asserts=False, num_devices=8,
    )
    xh_d = nc.dram_tensor("xh", [S, D], bf16, kind="ExternalInput").ap()
    xl_d = nc.dram_tensor("xl", [S, D], bf16, kind="ExternalInput").ap()
    wq_d = nc.dram_tensor("wq", [D, COLS], f32, kind="ExternalInput").ap()
    wk_d = nc.dram_tensor("wk", [D, COLS], f32, kind="ExternalInput").ap()
    wv_d = nc.dram_tensor("wv", [D, COLS], f32, kind="ExternalInput").ap()
    wo_d = nc.dram_tensor("wo", [COLS, D], f32, kind="ExternalInput").ap()
    bq_d = nc.dram_tensor("bq", [COLS], f32, kind="ExternalInput").ap()
    bk_d = nc.dram_tensor("bk", [COLS], f32, kind="ExternalInput").ap()
    out_d = nc.dram_tensor("out_t", [D, S], f32, kind="ExternalOutput").ap()

    with tile.TileContext(nc) as tc:
        with (
            tc.tile_pool(name="const", bufs=1) as const,
            tc.tile_pool(name="wst", bufs=1) as wst,
            tc.tile_pool(name="wpool", bufs=1) as wpool,
            tc.tile_pool(name="persist", bufs=1) as persist,
            tc.tile_pool(name="xhl", bufs=1) as xhl,
            tc.tile_pool(name="xtp", bufs=2) as xtp,
            tc.tile_pool(name="exps", bufs=2) as exps,
            tc.tile_pool(name="stage", bufs=3) as stage,
            tc.tile_pool(name="outst", bufs=4) as outst,
            tc.tile_pool(name="ps_sc", bufs=1, space="PSUM") as ps_sc,
            tc.tile_pool(name="ps_acc", bufs=2, space="PSUM") as ps_acc,
            tc.tile_pool(name="ps_u", bufs=2, space="PSUM") as ps_u,
        ):
            ones32 = const.tile([P, VW], f32, tag="ones32")
            nc.vector.memset(ones32[:], 1.0)
            ones_r = const.tile([P, VW], f32r, tag="ones_r")
            nc.vector.tensor_copy(ones_r[:], ones32[:])

            # ---- weights: DMA fp32 -> convert to f32r on DVE ----
            def load_w(dram, shape_free, name):
                st = wst.tile([P, KC, shape_free], f32, tag="wstage", name="wstage")
                nc.sync.dma_start(st[:], dram.rearrange("(o p) f -> p o f", p=P))
                wr = wpool.tile([P, KC, shape_free], f32r, tag=f"w_{name}",
                                name=f"w_{name}")
                nc.vector.tensor_copy(wr[:], st[:])
                return wr

            wq_r = load_w(wq_d, COLS, "q")
            wk_r = load_w(wk_d, COLS, "k")
            wv_r = load_w(wv_d, COLS, "v")
            wo_st = wst.tile([P, 2, D], f32, tag="wstage", name="wostage")
            nc.sync.dma_start(wo_st[:], wo_d.rearrange("(o p) f -> p o f", p=P))
            wo_r = wpool.tile([P, 2, D], f32r, tag="w_o")
            nc.vector.tensor_copy(wo_r[:], wo_st[:])

            bq_sb = const.tile([P, 2], f32, tag="bq")
            nc.sync.dma_start(bq_sb[:], bq_d.rearrange("(o p) -> p o", p=P))
            bk_sb = const.tile([P, 2], f32, tag="bk")
            nc.sync.dma_start(bk_sb[:], bk_d.rearrange("(o p) -> p o", p=P))

            # persistent activations
            qT = persist.tile([P, 2, S], f32r, tag="qT")    # [qcol, tok]
            kT = persist.tile([P, 2, S], f32r, tag="kT")    # [kcol, tok]
            vt = persist.tile([P, NT, 4 * VW], f32r, tag="vt")  # [tok, h*(1|V)]
            oT = persist.tile([P, 2, S], f32r, tag="oT")    # [vdim, tok]

            # ones column (index 64 of each head's VW slice)
            vt_heads = vt[:].rearrange("p t (h c) -> p t h c", c=VW)
            nc.vector.tensor_copy(
                vt_heads[:, :, :, 64],
                ones32[:, :NT * 4].rearrange("p (t h) -> p t h", h=4),
            )

            # ---- phase 0/1: x transpose + QKV projections, per token block ----
            for j in range(NJ):
                xT = xtp.tile([P, KC, TT], f32r, tag="xT")
                xth = xhl.tile([P, KC, TT], bf16, tag="xth", name="xth")
                nc.sync.dma_start_transpose(xth[:], xh_d[bass.ts(j, TT), :])
                xtl = xhl.tile([P, KC, TT], bf16, tag="xtl", name="xtl")
                nc.sync.dma_start_transpose(xtl[:], xl_d[bass.ts(j, TT), :])
                nc.vector.tensor_tensor(
                    xT[:], xth[:], xtl[:], mybir.AluOpType.add
                )

                # Q^T, K^T: [qcol, tok] with bias
                for (wmat, bsb, dstT) in ((wq_r, bq_sb, qT), (wk_r, bk_sb, kT)):
                    for ct in range(2):
                        acc = ps_u.tile([P, TT], f32, tag="u", name="qk_acc")
                        for kc in range(KC):
                            nc.tensor.matmul(
                                acc[:], wmat[:, kc, bass.ts(ct, P)], xT[:, kc, :],
                                start=(kc == 0), stop=(kc == KC - 1),
                            )
                        nc.vector.tensor_scalar_add(
                            dstT[:, ct, bass.ts(j, TT)], acc[:], bsb[:, ct : ct + 1]
                        )

                # V: [tok, vcol]
                for ts in range(TT // P):
                    acc = ps_u.tile([P, COLS], f32, tag="u", name="v_acc")
                    for kc in range(KC):
                        nc.tensor.matmul(
                            acc[:], xT[:, kc, bass.ts(ts, P)], wv_r[:, kc, :],
                            start=(kc == 0), stop=(kc == KC - 1),
                        )
                    tt = 4 * j + ts
                    nc.vector.tensor_copy(
                        vt_heads[:, tt, :, 0:64],
                        acc[:].rearrange("p (h c) -> p h c", c=64),
                    )

            # shared scores PSUM tensor: 4 slots x [128, 512] = 4 banks
            big_sc = ps_sc.tile([P, 4, TT], f32, tag="sc")

            # ---- phase 2 + 3 interleaved over token blocks ----
            for j in range(NJ):
                for p in range(2):
                    o_ps = [
                        ps_acc.tile([VW, TT], f32, tag="acc", name=f"o_ps{i}")
                        for i in range(2)
                    ]
                    for kc in range(NKT):
                        base = (2 * kc) % 4
                        for i in range(2):
                            lo, hi = 64 * i, 64 * i + 64
                            nc.tensor.matmul(
                                big_sc[:, base + i, :],
                                kT[lo:hi, p, bass.ts(kc, P)],
                                qT[lo:hi, p, bass.ts(j, TT)],
                                start=True, stop=True,
                            )
                        ex = exps.tile([P, 2, TT], f32r, tag="exp", name="ex")
                        nc.scalar.activation(
                            ex[:], big_sc[:, base : base + 2, :], Exp,
                            scale=0.125,
                        )
                        for i in range(2):
                            h = 2 * p + i
                            nc.tensor.matmul(
                                o_ps[i][:],
                                vt[:, kc, bass.ds(VW * h, VW)],
                                ex[:, i, :],
                                start=(kc == 0), stop=(kc == NKT - 1),
                            )

                    # normalize both heads into O^T via recip/broadcast/mult/DMA
                    o32 = stage.tile([P, TT], f32r, tag="o32", name="o32")
                    for i in range(2):
                        # free o_ps quickly: one copy to SBUF, then normalize
                        osb = stage.tile([P, TT], f32r, tag="osb", name="osb")
                        nc.vector.tensor_copy(osb[0:VW, :], o_ps[i][:])
                        # broadcast the sums row via PE ones outer-product
                        rbc = ps_u.tile([64, TT], f32, tag="u", name="rbc")
                        nc.tensor.matmul(
                            rbc[:], ones_r[64:65, 0:64], osb[64:65, :],
                            start=True, stop=True,
                        )
                        rbs = stage.tile([64, TT], f32, tag="rbs", name="rbs")
                        nc.vector.reciprocal_approx_fast(rbs[:], rbc[:])
                        onrm = stage.tile([P, TT], f32r, tag="onrm", name="onrm")
                        nc.vector.tensor_tensor(
                            onrm[0:64, :], osb[0:64, :], rbs[:],
                            mybir.AluOpType.mult,
                        )
                        nc.sync.dma_start(
                            o32[bass.ds(64 * i, 64), :], onrm[0:64, :]
                        )
                    nc.vector.tensor_copy(oT[:, p, bass.ts(j, TT)], o32[:])

                # partial output projection for this token block
                for oc in range(D // P):
                    acc = ps_u.tile([P, TT], f32, tag="u", name="wo_acc")
                    for vc in range(2):
                        nc.tensor.matmul(
                            acc[:], wo_r[:, vc, bass.ts(oc, P)],
                            oT[:, vc, bass.ts(j, TT)],
                            start=(vc == 0), stop=(vc == 1),
                        )
                    st = outst.tile([P, TT], f32, tag="outst", name="outst")
                    nc.vector.tensor_copy(st[:], acc[:])
                    nc.sync.dma_start(out_d[bass.ts(oc, P), bass.ts(j, TT)], st[:])

    nc.compile()
    return nc


def make_in_maps(x, Wq, bq, Wk, bk, Wv, Wo):
    import ml_dtypes

    xh = [None, None]
    xl = [None, None]
    for b in range(B):
        hi = x[b].astype(ml_dtypes.bfloat16)
        lo = (x[b] - hi.astype(np.float32)).astype(ml_dtypes.bfloat16)
        xh[b], xl[b] = np.ascontiguousarray(hi), np.ascontiguousarray(lo)

    in_maps = []
    for c in range(8):
        b, g = divmod(c, 4)
        cs = slice(COLS * g, COLS * (g + 1))
        in_maps.append({
            "xh": xh[b],
            "xl": xl[b],
            "wq": np.ascontiguousarray(Wq[:, cs]),
            "wk": np.ascontiguousarray(Wk[:, cs]),
            "wv": np.ascontiguousarray(Wv[:, cs]),
            "wo": np.ascontiguousarray(Wo[cs, :]),
            "bq": np.ascontiguousarray(bq[cs]),
            "bk": np.ascontiguousarray(bk[cs]),
        })
    return in_maps


def kernel(x, Wq, bq, Wk, bk, Wv, bv, Wo, bo):
    from concourse import bass_utils

    x = np.asarray(x, dtype=np.float32)
    Wq = np.asarray(Wq, dtype=np.float32)
    Wk = np.asarray(Wk, dtype=np.float32)
    Wv = np.asarray(Wv, dtype=np.float32)
    Wo = np.asarray(Wo, dtype=np.float32)
    bq = np.asarray(bq, dtype=np.float32)
    bk = np.asarray(bk, dtype=np.float32)
    bv = np.asarray(bv, dtype=np.float32)
    bo = np.asarray(bo, dtype=np.float32)

    if "nc" not in _CACHE:
        _CACHE["nc"] = _build()
    nc = _CACHE["nc"]

    in_maps = make_in_maps(x, Wq, bq, Wk, bk, Wv, Wo)
    res = bass_utils.run_bass_kernel_spmd(nc, in_maps, core_ids=list(range(8)))

    out = np.zeros((B, S, D), dtype=np.float32)
    for c in range(8):
        out[c // 4] += res.results[c]["out_t"].T
    out += bo + bv @ Wo
    return out


# revision 28
# speedup vs baseline: 1.1172x; 1.0723x over previous
"""Multi-head attention (B=2, S=2048, D=1024, H=16, dk=64) on 8 Trainium2
NeuronCores via Bass/Tile.

Sharding: core c handles batch b = c//4 and head-group g = c%4 (4 heads,
256 qkv columns).  Each core computes its QKV projection slices, 4 heads of
attention, and a partial output projection against its 256-row slice of Wo.
The host sums the 4 partial outputs per batch (row-sharded Wo => partial
sums) and folds in the biases bo and bv@Wo (softmax rows sum to 1, so the
V-bias contributes exactly bv@Wo per token).

v2 design notes:
- All matmuls in float32r (full-rate fp32 PE mode, ~1.5e-4 rounding); PE
  transposes also f32r (1.5 cyc/row) to avoid dtype switches.
- scoresT [k_tok, q_tok] per head via K=64 row-packed head pairs
  (tile_position (0,0)/(64,0) derived from base partitions) -> concurrent.
- Scores land in one shared 4-bank PSUM tensor [128, 8, 512]; ONE ACT exp
  per 2 k-chunks covers [128, 2048] (amortizes the 352-cycle ACT overhead).
- AV lhsT = [1 | V_h] so PSUM row 0 accumulates the softmax denominators.
- Normalization without PE transposes: DVE reciprocal of the sums row,
  PE ones-outer-product broadcast to [65, 512], DVE multiply, then a
  partition-shifting SBUF->SBUF DMA routes each head into O^T layout.
"""

import numpy as np

P = 128
B, S, D = 2, 2048, 1024
H, DK = 16, 64
COLS = 256          # qkv columns per core (4 heads)
KC = D // P         # 8 contraction chunks for the projections
TT = 512            # token block (matmul free dim)
NJ = S // TT        # 4 token blocks
NT = S // P         # 16 token tiles
NKT = S // P        # 16 key tiles
VW = 65             # per-head AV lhsT width: ones column + 64 v-dims

_CACHE = {}


def _build():
    import concourse.bass as bass
    import concourse.tile as tile
    from concourse import bacc, mybir

    f32 = mybir.dt.float32
    f32r = mybir.dt.float32r
    Exp = mybir.ActivationFunctionType.Exp

    bf16 = mybir.dt.bfloat16
    nc = bacc.Bacc(
        "TRN2", target_bir_lowering=False, debug=False,
        enable_asserts=False, num_devices=8,
    )
    xh_d = nc.dram_tensor("xh", [S, D], bf16, kind="ExternalInput").ap()
    xl_d = nc.dram_tensor("xl", [S, D], bf16, kind="ExternalInput").ap()
    wq_d = nc.dram_tensor("wq", [D, COLS], f32, kind="ExternalInput").ap()
    wk_d = nc.dram_tensor("wk", [D, COLS], f32, kind="ExternalInput").ap()
    wv_d = nc.dram_tensor("wv", [D, COLS], f32, kind="ExternalInput").ap()
    wo_d = nc.dram_tensor("wo", [COLS, D], f32, kind="ExternalInput").ap()
    bq_d = nc.dram_tensor("bq", [COLS], f32, kind="ExternalInput").ap()
    bk_d = nc.dram_tensor("bk", [COLS], f32, kind="ExternalInput").ap()
    out_d = nc.dram_tensor("out_t", [D, S], f32, kind="ExternalOutput").ap()

    with tile.TileContext(nc) as tc:
        with (
            tc.tile_pool(name="const", bufs=1) as const,
            tc.tile_pool(name="wst", bufs=1) as wst,
            tc.tile_pool(name="wpool", bufs=1) as wpool,
            tc.tile_pool(name="persist", bufs=1) as persist,
            tc.tile_pool(name="xhl", bufs=1) as xhl,
            tc.tile_pool(name="xtp", bufs=2) as xtp,
            tc.tile_pool(name="exps", bufs=3) as exps,
            tc.tile_pool(name="stage", bufs=3) as stage,
            tc.tile_pool(name="outst", bufs=4) as outst,
            tc.tile_pool(name="ps_sc", bufs=1, space="PSUM") as ps_sc,
            tc.tile_pool(name="ps_acc", bufs=2, space="PSUM") as ps_acc,
        ):
            ones32 = const.tile([P, VW], f32, tag="ones32")
            nc.vector.memset(ones32[:], 1.0)
            ones_r = const.tile([P, VW], f32r, tag="ones_r")
            nc.vector.tensor_copy(ones_r[:], ones32[:])

            # ---- weights: DMA fp32 -> convert to f32r on DVE ----
            def load_w(dram, shape_free, name):
                st = wst.tile([P, KC, shape_free], f32, tag="wstage", name="wstage")
                nc.sync.dma_start(st[:], dram.rearrange("(o p) f -> p o f", p=P))
                wr = wpool.tile([P, KC, shape_free], f32r, tag=f"w_{name}",
                                name=f"w_{name}")
                nc.vector.tensor_copy(wr[:], st[:])
                return wr

            wq_r = load_w(wq_d, COLS, "q")
            wk_r = load_w(wk_d, COLS, "k")
            wv_r = load_w(wv_d, COLS, "v")
            wo_st = wst.tile([P, 2, D], f32, tag="wstage", name="wostage")
            nc.sync.dma_start(wo_st[:], wo_d.rearrange("(o p) f -> p o f", p=P))
            wo_r = wpool.tile([P, 2, D], f32r, tag="w_o")
            nc.vector.tensor_copy(wo_r[:], wo_st[:])

            bq_sb = const.tile([P, 2], f32, tag="bq")
            nc.sync.dma_start(bq_sb[:], bq_d.rearrange("(o p) -> p o", p=P))
            bk_sb = const.tile([P, 2], f32, tag="bk")
            nc.sync.dma_start(bk_sb[:], bk_d.rearrange("(o p) -> p o", p=P))

            # persistent activations
            qT = persist.tile([P, 2, S], f32r, tag="qT")    # [qcol, tok]
            kT = persist.tile([P, 2, S], f32r, tag="kT")    # [kcol, tok]
            vt = persist.tile([P, NT, 4 * VW], f32r, tag="vt")  # [tok, h*(1|V)]
            oT = persist.tile([P, 2, S], f32r, tag="oT")    # [vdim, tok]

            # ones column (index 64 of each head's VW slice)
            vt_heads = vt[:].rearrange("p t (h c) -> p t h c", c=VW)
            nc.vector.tensor_copy(
                vt_heads[:, :, :, 64],
                ones32[:, :NT * 4].rearrange("p (t h) -> p t h", h=4),
            )

            # shared PSUM tensor: 6 slots x [128, 512] = 6 banks.  Scores
            # pipeline uses slot pairs (2kc)%6; phase-1 QKV and the final Wo
            # accumulations borrow slots round-robin (Tile range-deps keep it
            # sound).
            big_sc = ps_sc.tile([P, 6, TT], f32, tag="sc")
            slot_rr = [0]

            def next_slot():
                s = slot_rr[0] % 6
                slot_rr[0] += 1
                return s

            # ---- phase 0/1: x transpose + QKV projections, per token block ----
            for j in range(NJ):
                xT = xtp.tile([P, KC, TT], f32r, tag="xT")
                xth = xhl.tile([P, KC, TT], bf16, tag="xth", name="xth")
                nc.sync.dma_start_transpose(xth[:], xh_d[bass.ts(j, TT), :])
                xtl = xhl.tile([P, KC, TT], bf16, tag="xtl", name="xtl")
                nc.sync.dma_start_transpose(xtl[:], xl_d[bass.ts(j, TT), :])
                nc.vector.tensor_tensor(
                    xT[:], xth[:], xtl[:], mybir.AluOpType.add
                )

                # Q^T, K^T: [qcol, tok] with bias
                for (wmat, bsb, dstT) in ((wq_r, bq_sb, qT), (wk_r, bk_sb, kT)):
                    for ct in range(2):
                        acc = big_sc[:, next_slot(), :]
                        for kc in range(KC):
                            nc.tensor.matmul(
                                acc, wmat[:, kc, bass.ts(ct, P)], xT[:, kc, :],
                                start=(kc == 0), stop=(kc == KC - 1),
                            )
                        nc.vector.tensor_scalar_add(
                            dstT[:, ct, bass.ts(j, TT)], acc, bsb[:, ct : ct + 1]
                        )

                # V: [tok, vcol]
                for ts in range(TT // P):
                    acc = big_sc[:, next_slot(), 0:COLS]
                    for kc in range(KC):
                        nc.tensor.matmul(
                            acc, xT[:, kc, bass.ts(ts, P)], wv_r[:, kc, :],
                            start=(kc == 0), stop=(kc == KC - 1),
                        )
                    tt = 4 * j + ts
                    nc.vector.tensor_copy(
                        vt_heads[:, tt, :, 0:64],
                        acc.rearrange("p (h c) -> p h c", c=64),
                    )

            # ---- phase 2: attention units ----
            for j in range(NJ):
                for p in range(2):
                    o_ps = [
                        ps_acc.tile([VW, TT], f32, tag="acc", name=f"o_ps{i}")
                        for i in range(2)
                    ]
                    # software-pipelined emission: scores run 2 k-chunks ahead,
                    # AV trails exp by one, so PE always has ready work while
                    # ACT's ~1.1us exp latency is in flight.
                    def sc_emit(kc):
                        base = (2 * kc) % 6
                        for i in range(2):
                            lo, hi = 64 * i, 64 * i + 64
                            nc.tensor.matmul(
                                big_sc[:, base + i, :],
                                kT[lo:hi, p, bass.ts(kc, P)],
                                qT[lo:hi, p, bass.ts(j, TT)],
                                start=True, stop=True,
                            )

                    def av_emit(kc, ex):
                        for i in range(2):
                            h = 2 * p + i
                            nc.tensor.matmul(
                                o_ps[i][:],
                                vt[:, kc, bass.ds(VW * h, VW)],
                                ex[:, i, :],
                                start=(kc == 0), stop=(kc == NKT - 1),
                            )

                    sc_emit(0)
                    sc_emit(1)
                    sc_emit(2)
                    prev = None
                    for kc in range(NKT):
                        base = (2 * kc) % 6
                        ex = exps.tile([P, 2, TT], f32r, tag="exp", name="ex")
                        nc.scalar.activation(
                            ex[:], big_sc[:, base : base + 2, :], Exp,
                            scale=0.125,
                        )
                        if prev is not None:
                            av_emit(kc - 1, prev)
                        if kc + 3 < NKT:
                            sc_emit(kc + 3)
                        prev = ex
                    av_emit(NKT - 1, prev)

                    # normalize both heads into O^T via recip/broadcast/mult/DMA
                    o32 = stage.tile([P, TT], f32r, tag="o32", name="o32")
                    for i in range(2):
                        # free o_ps quickly: one copy to SBUF, then normalize
                        osb = stage.tile([P, TT], f32r, tag="osb", name="osb")
                        nc.vector.tensor_copy(osb[0:VW, :], o_ps[i][:])
                        # broadcast the sums row via PE ones outer-product,
                        # writing back into the (now consumed) o_ps tile
                        nc.tensor.matmul(
                            o_ps[i][0:64, :], ones_r[64:65, 0:64], osb[64:65, :],
                            start=True, stop=True,
                        )
                        rbs = stage.tile([64, TT], f32, tag="rbs", name="rbs")
                        nc.vector.reciprocal_approx_fast(rbs[:], o_ps[i][0:64, :])
                        onrm = stage.tile([P, TT], f32r, tag="onrm", name="onrm")
                        nc.vector.tensor_tensor(
                            onrm[0:64, :], osb[0:64, :], rbs[:],
                            mybir.AluOpType.mult,
                        )
                        nc.sync.dma_start(
                            o32[bass.ds(64 * i, 64), :], onrm[0:64, :]
                        )
                    nc.vector.tensor_copy(oT[:, p, bass.ts(j, TT)], o32[:])

            # ---- phase 3: partial output projection (scheduler gap-fills) ----
            for j in range(NJ):
                for oc in range(D // P):
                    acc = big_sc[:, next_slot(), :]
                    for vc in range(2):
                        nc.tensor.matmul(
                            acc, wo_r[:, vc, bass.ts(oc, P)],
                            oT[:, vc, bass.ts(j, TT)],
                            start=(vc == 0), stop=(vc == 1),
                        )
                    st = outst.tile([P, TT], f32, tag="outst", name="outst")
                    nc.vector.tensor_copy(st[:], acc)
                    nc.sync.dma_start(out_d[bass.ts(oc, P), bass.ts(j, TT)], st[:])

    nc.compile()
    return nc


def make_in_maps(x, Wq, bq, Wk, bk, Wv, Wo):
    import ml_dtypes

    xh = [None, None]
    xl = [None, None]
    for b in range(B):
        hi = x[b].astype(ml_dtypes.bfloat16)
        lo = (x[b] - hi.astype(np.float32)).astype(ml_dtypes.bfloat16)
        xh[b], xl[b] = np.ascontiguousarray(hi), np.ascontiguousarray(lo)

    in_maps = []
    for c in range(8):
        b, g = divmod(c, 4)
        cs = slice(COLS * g, COLS * (g + 1))
        in_maps.append({
            "xh": xh[b],
            "xl": xl[b],
            "wq": np.ascontiguousarray(Wq[:, cs]),
            "wk": np.ascontiguousarray(Wk[:, cs]),
            "wv": np.ascontiguousarray(Wv[:, cs]),
            "wo": np.ascontiguousarray(Wo[cs, :]),
            "bq": np.ascontiguousarray(bq[cs]),
            "bk": np.ascontiguousarray(bk[cs]),
        })
    return in_maps


def kernel(x, Wq, bq, Wk, bk, Wv, bv, Wo, bo):
    from concourse import bass_utils

    x = np.asarray(x, dtype=np.float32)
    Wq = np.asarray(Wq, dtype=np.float32)
    Wk = np.asarray(Wk, dtype=np.float32)
    Wv = np.asarray(Wv, dtype=np.float32)
    Wo = np.asarray(Wo, dtype=np.float32)
    bq = np.asarray(bq, dtype=np.float32)
    bk = np.asarray(bk, dtype=np.float32)
    bv = np.asarray(bv, dtype=np.float32)
    bo = np.asarray(bo, dtype=np.float32)

    if "nc" not in _CACHE:
        _CACHE["nc"] = _build()
    nc = _CACHE["nc"]

    in_maps = make_in_maps(x, Wq, bq, Wk, bk, Wv, Wo)
    res = bass_utils.run_bass_kernel_spmd(nc, in_maps, core_ids=list(range(8)))

    out = np.zeros((B, S, D), dtype=np.float32)
    for c in range(8):
        out[c // 4] += res.results[c]["out_t"].T
    out += bo + bv @ Wo
    return out
